# revision 1
# baseline (speedup 1.0000x reference)
"""ConcatCritic pair-grid MLP on 8 TRN2 NeuronCores.

Computes out[i, j] = f(x[i], y[j]) where f is a 3-hidden-layer MLP over the
concatenated pair, decomposed so the first layer is two small projections
summed by broadcast (no [B, B, A+B] concat tensor).

Sharding: the B^2 pair grid is split row-wise (x batch) across 8 cores;
y and all MLP parameters are replicated. Each core produces a [B/8, B]
score tile; the host concatenates them. b3 (a scalar) is added on the host.

Device layout: activations live transposed as [hid-on-partitions, pairs-on-
free] so every layer matmul is lhsT=W_block [128(k),128(m)], rhs=hT
[128(k), 512(pairs)] accumulating over 4 k-blocks into PSUM. Matmul operands
are float32r (fp22 multiply at full PE rate, fp32 accumulate). PSUM->SBUF
relu+bias drains are split between ScalarE and VectorE to keep both under
the TensorE span.
"""

import numpy as np

import concourse.bass as bass
import concourse.mybir as mybir
from concourse import bacc
from concourse.bass_utils import run_bass_kernel_spmd
from concourse.tile import TileContext

B = 256
A_DIM = 128
HID = 512
N_CORES = 8
ROWS = B // N_CORES  # 32 x-rows per core
KB = HID // 128  # 4 k-blocks of 128
PAIR_TILE = 512  # pairs per matmul tile = 2 x-rows x 256 y-rows
ROWS_PER_TILE = PAIR_TILE // B  # 2
N_TILES = ROWS // ROWS_PER_TILE  # 16

F32 = mybir.dt.float32
F32R = mybir.dt.float32r

_CACHE = {}


def _build_nc():
    nc = bacc.Bacc()

    xT = nc.declare_dram_parameter("xT", [A_DIM, ROWS], F32R, isOutput=False)
    yT = nc.declare_dram_parameter("yT", [A_DIM, B], F32R, isOutput=False)
    Wx = nc.declare_dram_parameter("Wx", [A_DIM, HID], F32R, isOutput=False)
    Wy = nc.declare_dram_parameter("Wy", [A_DIM, HID], F32R, isOutput=False)
    W1 = nc.declare_dram_parameter("W1", [HID, HID], F32R, isOutput=False)
    W2 = nc.declare_dram_parameter("W2", [HID, HID], F32R, isOutput=False)
    W3 = nc.declare_dram_parameter("W3", [HID, 1], F32R, isOutput=False)
    b0r = nc.declare_dram_parameter("b0r", [128, KB], F32, isOutput=False)
    b1r = nc.declare_dram_parameter("b1r", [128, KB], F32, isOutput=False)
    b2r = nc.declare_dram_parameter("b2r", [128, KB], F32, isOutput=False)
    out = nc.declare_dram_parameter("out", [1, ROWS * B], F32, isOutput=True)

    relu = mybir.ActivationFunctionType.Relu

    with TileContext(nc) as tc:
        with (
            tc.tile_pool(name="const", bufs=1) as const,
            tc.tile_pool(name="work", bufs=3) as work,
            tc.tile_pool(name="sc_pool", bufs=4) as sc_pool,
            tc.tile_pool(name="psum", bufs=6, space="PSUM") as psum,
            tc.tile_pool(name="psum_s", bufs=2, space="PSUM") as psum_s,
        ):
            # ---- load replicated constants -------------------------------
            xT_sb = const.tile([A_DIM, ROWS], F32R)
            yT_sb = const.tile([A_DIM, B], F32R)
            Wx_sb = const.tile([A_DIM, HID], F32R)
            Wy_sb = const.tile([A_DIM, HID], F32R)
            b0_sb = const.tile([128, KB], F32)
            b1_sb = const.tile([128, KB], F32)
            b2_sb = const.tile([128, KB], F32)
            W1_sb = const.tile([128, KB, HID], F32R)
            W2_sb = const.tile([128, KB, HID], F32R)
            W3_sb = const.tile([128, KB, 1], F32R)

            nc.sync.dma_start(xT_sb[:], xT[:, :])
            nc.sync.dma_start(Wx_sb[:], Wx[:, :])
            nc.sync.dma_start(yT_sb[:], yT[:, :])
            nc.sync.dma_start(Wy_sb[:], Wy[:, :])
            nc.sync.dma_start(b0_sb[:], b0r[:, :])
            # W1 chunks before anything L2 needs: tile-0 layer-1 k-group
            # matmuls gate on W1 k-block arrival.
            w1_r = W1[:, :].rearrange("(k p) n -> p k n", p=128)
            w2_r = W2[:, :].rearrange("(k p) n -> p k n", p=128)
            for k in range(KB):
                nc.sync.dma_start(W1_sb[:, k], w1_r[:, k])
            nc.sync.dma_start(b1_sb[:], b1r[:, :])
            for k in range(KB):
                nc.sync.dma_start(W2_sb[:, k], w2_r[:, k])
            nc.sync.dma_start(b2_sb[:], b2r[:, :])
            nc.sync.dma_start(W3_sb[:], W3[:, :].rearrange("(k p) n -> p k n", p=128))

            # ---- input projections --------------------------------------
            # bxT[p, m, i] = (x @ Wx)^T[m*128+p, i] + b0[m*128+p]
            # hx/hy interleaved per block m and drains split DVE/ACT so the
            # first pair-tile's layer-0 (DVE) and layer-1 (PE) start early.
            bxT = const.tile([128, KB, ROWS], F32)
            hyT = const.tile([128, KB, B], F32)
            for m in range(KB):
                sl = slice(m * 128, (m + 1) * 128)
                ph = psum.tile([128, PAIR_TILE], F32, tag="ps", name="ph")[:, :ROWS]
                nc.tensor.matmul(ph, Wx_sb[:, sl], xT_sb[:], start=True, stop=True)
                nc.vector.tensor_scalar_add(bxT[:, m], ph, b0_sb[:, m : m + 1])
                ph2 = psum.tile([128, PAIR_TILE], F32, tag="ps", name="ph2")[:, :B]
                nc.tensor.matmul(ph2, Wy_sb[:, sl], yT_sb[:], start=True, stop=True)
                nc.scalar.copy(out=hyT[:, m], in_=ph2)

            # ---- main pair-tile loop ------------------------------------
            for t in range(N_TILES):
                i0 = t * ROWS_PER_TILE
                # layer 0 on DVE (SBUF->SBUF is cheap there):
                # h0T[p, k, a*256+j] = relu(hyT[p,k,j] + bxT[p,k,i0+a])
                h0T = work.tile([128, KB, PAIR_TILE], F32R, tag="h0")
                for k in range(KB):
                    for a in range(ROWS_PER_TILE):
                        nc.vector.tensor_scalar(
                            h0T[:, k, a * B : (a + 1) * B],
                            hyT[:, k],
                            bxT[:, k, i0 + a : i0 + a + 1],
                            0.0,
                            mybir.AluOpType.add,
                            mybir.AluOpType.max,
                        )
                # layers 1 and 2; PSUM drains (relu+bias) split 5:3 between
                # ScalarE and VectorE so both stay under the TensorE span.
                hin = h0T
                for layer, (W_sb, b_sb) in enumerate(((W1_sb, b1_sb), (W2_sb, b2_sb))):
                    hout = work.tile([128, KB, PAIR_TILE], F32R, tag=f"h{layer + 1}")
                    for m in range(KB):
                        pt = psum.tile([128, PAIR_TILE], F32, tag="ps", name="pt")
                        for k in range(KB):
                            nc.tensor.matmul(
                                pt,
                                W_sb[:, k, m * 128 : (m + 1) * 128],
                                hin[:, k],
                                start=(k == 0),
                                stop=(k == KB - 1),
                            )
                        on_act = (m % 2 == 0) if layer == 0 else (m != 3)
                        if on_act:
                            nc.scalar.activation(
                                hout[:, m],
                                pt,
                                relu,
                                bias=b_sb[:, m : m + 1],
                                scale=1.0,
                            )
                        else:
                            nc.vector.tensor_scalar(
                                hout[:, m],
                                pt,
                                b_sb[:, m : m + 1],
                                0.0,
                                mybir.AluOpType.add,
                                mybir.AluOpType.max,
                            )
                    hin = hout
                # layer 3: [1, 512] scores for this tile (b3 added on host)
                ps = psum_s.tile([128, PAIR_TILE], F32, tag="sc", name="ps")[:1]
                for k in range(KB):
                    nc.tensor.matmul(
                        ps,
                        W3_sb[:, k],
                        hin[:, k],
                        start=(k == 0),
                        stop=(k == KB - 1),
                    )
                sc_sb = sc_pool.tile([1, PAIR_TILE], F32, tag="sc_sb")
                nc.scalar.copy(out=sc_sb[:], in_=ps)
                nc.sync.dma_start(
                    out[:, t * PAIR_TILE : (t + 1) * PAIR_TILE], sc_sb[:]
                )

    nc.compile()
    return nc


def _get_nc():
    if "nc" not in _CACHE:
        _CACHE["nc"] = _build_nc()
    return _CACHE["nc"]


def _prep_in_maps(inputs):
    f = lambda a: np.ascontiguousarray(np.asarray(a), dtype=np.float32)
    x, y = f(inputs["x"]), f(inputs["y"])
    shared = {
        "yT": f(y.T),
        "Wx": f(inputs["Wx"]),
        "Wy": f(inputs["Wy"]),
        "W1": f(inputs["W1"]),
        "W2": f(inputs["W2"]),
        "W3": f(inputs["W3"]),
        "b0r": f(np.asarray(inputs["b0"]).reshape(KB, 128).T),
        "b1r": f(np.asarray(inputs["b1"]).reshape(KB, 128).T),
        "b2r": f(np.asarray(inputs["b2"]).reshape(KB, 128).T),
    }
    in_maps = []
    for m in range(N_CORES):
        im = dict(shared)
        im["xT"] = f(x[m * ROWS : (m + 1) * ROWS].T)
        in_maps.append(im)
    return in_maps


def run(trace=False, **inputs):
    nc = _get_nc()
    in_maps = _prep_in_maps(inputs)
    res = run_bass_kernel_spmd(nc, in_maps, core_ids=list(range(N_CORES)), trace=trace)
    b3 = np.float32(np.asarray(inputs["b3"]).reshape(-1)[0])
    blocks = [r["out"].reshape(ROWS, B) + b3 for r in res.results]
    return np.concatenate(blocks, axis=0).astype(np.float32), res


def kernel(**inputs):
    out, _ = run(trace=False, **inputs)
    return out



# revision 3
# speedup vs baseline: 1.2121x; 1.2121x over previous
"""ConcatCritic pair-grid MLP on 8 TRN2 NeuronCores — fp8 DoubleRow version.

Computes out[i, j] = f(x[i], y[j]) where f is a 3-hidden-layer MLP over the
concatenated pair, decomposed so the first layer is two small projections
summed by broadcast.

Sharding: the B^2 pair grid is split row-wise (x batch) across 8 cores;
y and all MLP parameters are replicated. Each core produces a [B/8, B]
score tile; the host concatenates them. b3 (a scalar) is added on the host.

Numerics: layers 1-2 run on the PE array in fp8e4m3 with DoubleRow perf
mode (two 128-row k-groups per instruction at 0.5 cycles/row). Activations
are stored as single fp8 with power-of-2 scales; W1 is split hi+lo into two
fp8 products to cut its quantization error; W2 is a single fp8 product;
layer 3 runs in bf16. Each layer's bias is folded into one extra DoubleRow
matmul against a constant ones tile (the bias is hi/lo-encoded across 64
lhsT slots), which makes every PSUM drain a pure scale+relu op that DVE and
ACT can both execute, including paired two-bank drains.
"""

import numpy as np
import ml_dtypes

import concourse.bass as bass
import concourse.mybir as mybir
from concourse import bacc
from concourse.bass_utils import run_bass_kernel_spmd
from concourse.tile import TileContext

B = 256
A_DIM = 128
HID = 512
N_CORES = 8
ROWS = B // N_CORES  # 32 x-rows per core
KB = HID // 128  # 4 k-blocks of 128
PAIR = 512  # pairs per tile = 2 x-rows x 256 y-rows
ROWS_PER_TILE = PAIR // B  # 2
N_TILES = ROWS // ROWS_PER_TILE  # 16

F32 = mybir.dt.float32
F32R = mybir.dt.float32r
F8 = mybir.dt.float8e4
BF16 = mybir.dt.bfloat16
E4 = ml_dtypes.float8_e4m3
BF = ml_dtypes.bfloat16
DR = mybir.MatmulPerfMode.DoubleRow

_CACHE = {}


def _build_nc(sc1, sc2, sc3):
    nc = bacc.Bacc()

    xT = nc.declare_dram_parameter("xT", [A_DIM, ROWS], F32R, isOutput=False)
    yT = nc.declare_dram_parameter("yT", [A_DIM, B], F32R, isOutput=False)
    Wx = nc.declare_dram_parameter("Wx", [A_DIM, HID], F32R, isOutput=False)
    Wy = nc.declare_dram_parameter("Wy", [A_DIM, HID], F32R, isOutput=False)
    b0s = nc.declare_dram_parameter("b0s", [128, KB], F32, isOutput=False)
    W1hi = nc.declare_dram_parameter("W1hi", [128, KB, HID], F8, isOutput=False)
    W1lo = nc.declare_dram_parameter("W1lo", [128, KB, HID], F8, isOutput=False)
    W2q = nc.declare_dram_parameter("W2q", [128, KB, HID], F8, isOutput=False)
    W3b = nc.declare_dram_parameter("W3b", [128, KB, 1], BF16, isOutput=False)
    bias1L = nc.declare_dram_parameter("bias1L", [128, 2, HID], F8, isOutput=False)
    bias2L = nc.declare_dram_parameter("bias2L", [128, 2, HID], F8, isOutput=False)
    ones8 = nc.declare_dram_parameter("ones8", [128, 2, PAIR], F8, isOutput=False)
    out = nc.declare_dram_parameter("out", [1, ROWS * B], F32, isOutput=True)

    relu = mybir.ActivationFunctionType.Relu
    copyf = mybir.ActivationFunctionType.Copy
    add = mybir.AluOpType.add
    amax = mybir.AluOpType.max
    mult = mybir.AluOpType.mult

    with TileContext(nc) as tc:
        with (
            tc.tile_pool(name="const", bufs=1) as const,
            tc.tile_pool(name="h0p", bufs=2) as h0p,
            tc.tile_pool(name="h1p", bufs=2) as h1p,
            tc.tile_pool(name="h2p", bufs=2) as h2p,
            tc.tile_pool(name="scp", bufs=4) as scp,
            tc.tile_pool(name="psA", bufs=3, space="PSUM") as psA,
            tc.tile_pool(name="psS", bufs=1, space="PSUM") as psS,
        ):
            # ---- replicated constants -----------------------------------
            W1hi_sb = const.tile([128, KB, HID], F8)
            W1lo_sb = const.tile([128, KB, HID], F8)
            W2q_sb = const.tile([128, KB, HID], F8)
            W3b_sb = const.tile([128, KB, 1], BF16)
            bias1_sb = const.tile([128, 2, HID], F8)
            bias2_sb = const.tile([128, 2, HID], F8)
            ones_sb = const.tile([128, 2, PAIR], F8)
            xT_sb = const.tile([A_DIM, ROWS], F32R)
            yT_sb = const.tile([A_DIM, B], F32R)
            Wx_sb = const.tile([A_DIM, HID], F32R)
            Wy_sb = const.tile([A_DIM, HID], F32R)
            b0s_sb = const.tile([128, KB], F32)

            nc.sync.dma_start(xT_sb[:], xT[:, :])
            nc.sync.dma_start(Wx_sb[:], Wx[:, :])
            nc.sync.dma_start(yT_sb[:], yT[:, :])
            nc.sync.dma_start(Wy_sb[:], Wy[:, :])
            nc.sync.dma_start(b0s_sb[:], b0s[:, :])
            nc.sync.dma_start(W1hi_sb[:], W1hi[:, :, :])
            nc.sync.dma_start(ones_sb[:], ones8[:, :, :])
            nc.sync.dma_start(bias1_sb[:], bias1L[:, :, :])
            nc.sync.dma_start(W1lo_sb[:], W1lo[:, :, :])
            nc.sync.dma_start(W2q_sb[:], W2q[:, :, :])
            nc.sync.dma_start(bias2_sb[:], bias2L[:, :, :])
            nc.sync.dma_start(W3b_sb[:], W3b[:, :, :])

            # ---- input projections (f32r, exact) ------------------------
            # bxT[p, m, i] = (x*s0 @ Wx)^T + b0*s0 (f32; scalar operands)
            # hyT[p, m, j] = (y*s0 @ Wy)^T (bf16)
            bxT = const.tile([128, KB, ROWS], F32)
            hyT = const.tile([128, KB, B], BF16)
            for m in range(KB):
                sl = slice(m * 128, (m + 1) * 128)
                ph = psS.tile([128, PAIR], F32, tag="ps_s", name="ph")[:, :ROWS]
                nc.tensor.matmul(ph, Wx_sb[:, sl], xT_sb[:], start=True, stop=True)
                nc.vector.tensor_scalar(
                    bxT[:, m], ph, b0s_sb[:, m : m + 1], None, add
                )
                ph2 = psS.tile([128, PAIR], F32, tag="ps_s", name="ph2")[:, :B]
                nc.tensor.matmul(ph2, Wy_sb[:, sl], yT_sb[:], start=True, stop=True)
                nc.scalar.copy(out=hyT[:, m], in_=ph2)

            # ---- main pair-tile loop ------------------------------------
            for t in range(N_TILES):
                i0 = t * ROWS_PER_TILE
                # layer 0: h0q[p,k,a*256+j] = fp8(relu(hyT[p,k,j] + bxT[p,k,i0+a]))
                h0q = h0p.tile([128, KB, PAIR], F8, tag="h0")
                for k in range(KB):
                    for a in range(ROWS_PER_TILE):
                        dst = h0q[:, k, a * B : (a + 1) * B]
                        bxc = bxT[:, k, i0 + a : i0 + a + 1]
                        if k == 3 and a == 0:
                            nc.vector.tensor_scalar(
                                dst, hyT[:, k], bxc, 0.0, add, amax
                            )
                        elif k == 3 and a == 1:
                            nc.scalar.activation(
                                dst, hyT[:, k], relu, bias=bxc, scale=1.0
                            )
                        else:
                            nc.gpsimd.tensor_scalar(
                                dst, hyT[:, k], bxc, 0.0, add, amax
                            )

                # layer 1: fp8 DoubleRow, W1 hi+lo products + bias matmul
                h1q = h1p.tile([128, KB, PAIR], F8, tag="h1")
                for mp in range(2):
                    pt = psA.tile([128, 2, PAIR], F32, tag="psA", name="pt")
                    for h in range(2):
                        m = 2 * mp + h
                        msl = slice(m * 128, (m + 1) * 128)
                        nc.tensor.matmul(
                            pt[:, h], W1hi_sb[:, 0:2, msl], h0q[:, 0:2, :],
                            start=True, stop=False, perf_mode=DR,
                        )
                        nc.tensor.matmul(
                            pt[:, h], W1hi_sb[:, 2:4, msl], h0q[:, 2:4, :],
                            start=False, stop=False, perf_mode=DR,
                        )
                        nc.tensor.matmul(
                            pt[:, h], W1lo_sb[:, 0:2, msl], h0q[:, 0:2, :],
                            start=False, stop=False, perf_mode=DR,
                        )
                        nc.tensor.matmul(
                            pt[:, h], W1lo_sb[:, 2:4, msl], h0q[:, 2:4, :],
                            start=False, stop=False, perf_mode=DR,
                        )
                        nc.tensor.matmul(
                            pt[:, h], bias1_sb[:, :, msl], ones_sb[:],
                            start=False, stop=True, perf_mode=DR,
                        )
                    dst = h1q[:, 2 * mp : 2 * mp + 2, :]
                    if mp == 0:
                        nc.vector.tensor_scalar(dst, pt, sc1, 0.0, mult, amax)
                    else:
                        nc.scalar.activation(dst, pt, relu, bias=0.0, scale=sc1)

                # layer 2: fp8 DoubleRow, single W2 product + bias matmul
                h2q = h2p.tile([128, KB, PAIR], BF16, tag="h2")
                for mp in range(2):
                    pt = psA.tile([128, 2, PAIR], F32, tag="psA", name="pt2")
                    for h in range(2):
                        m = 2 * mp + h
                        msl = slice(m * 128, (m + 1) * 128)
                        nc.tensor.matmul(
                            pt[:, h], W2q_sb[:, 0:2, msl], h1q[:, 0:2, :],
                            start=True, stop=False, perf_mode=DR,
                        )
                        nc.tensor.matmul(
                            pt[:, h], W2q_sb[:, 2:4, msl], h1q[:, 2:4, :],
                            start=False, stop=False, perf_mode=DR,
                        )
                        nc.tensor.matmul(
                            pt[:, h], bias2_sb[:, :, msl], ones_sb[:],
                            start=False, stop=True, perf_mode=DR,
                        )
                    dst = h2q[:, 2 * mp : 2 * mp + 2, :]
                    if mp == 0:
                        nc.vector.tensor_scalar(dst, pt, sc2, 0.0, mult, amax)
                    else:
                        nc.scalar.activation(dst, pt, relu, bias=0.0, scale=sc2)

                # layer 3: bf16 matmul -> [1, 512] scores
                ps3 = psS.tile([128, PAIR], F32, tag="ps_s", name="ps3")[:1]
                for k in range(KB):
                    nc.tensor.matmul(
                        ps3, W3b_sb[:, k], h2q[:, k],
                        start=(k == 0), stop=(k == KB - 1),
                    )
                sc_sb = scp.tile([1, PAIR], F32, tag="sc_sb")
                nc.scalar.activation(sc_sb[:], ps3, copyf, bias=0.0, scale=sc3)
                nc.sync.dma_start(out[:, t * PAIR : (t + 1) * PAIR], sc_sb[:])

    nc.compile()
    return nc


def _q8(a):
    return np.asarray(a, np.float32).astype(E4)


def _p2(m, target=112.0):
    return float(2.0 ** np.floor(np.log2(target / m)))


def _prep(inputs):
    """Host-side quantization; returns (scales, shared in_map, per-core xT)."""
    f = lambda a: np.ascontiguousarray(np.asarray(a), dtype=np.float32)
    x, y = f(inputs["x"]), f(inputs["y"])
    Wx, Wy, b0 = f(inputs["Wx"]), f(inputs["Wy"]), f(inputs["b0"])
    W1, b1 = f(inputs["W1"]), f(inputs["b1"])
    W2, b2 = f(inputs["W2"]), f(inputs["b2"])
    W3 = f(inputs["W3"])

    hx = x @ Wx
    hy = y @ Wy
    h0max = float(np.max(np.max(hx + b0, 0) + np.max(hy, 0)))
    s0 = _p2(h0max)
    # subsampled forward for h1/h2 ranges (16 x-rows), 2x margin
    h0s = np.maximum(hx[::16][:, None, :] + hy[None, :, :] + b0, 0)
    h1s = np.maximum(h0s @ W1 + b1, 0)
    h2s = np.maximum(h1s @ W2 + b2, 0)
    s1 = _p2(float(np.max(h1s)) * 2) * 2
    s2 = _p2(float(np.max(h2s)) * 2) * 2
    t1 = _p2(float(np.max(np.abs(W1))))
    t2 = _p2(float(np.max(np.abs(W2))))
    t3 = _p2(float(np.max(np.abs(W3))))

    def kmajor(W):  # [HID, N] -> [128, KB, N]
        return np.ascontiguousarray(W.reshape(KB, 128, -1).transpose(1, 0, 2))

    W1hi_f = _q8(W1 * t1).astype(np.float32)
    W1hi = _q8(kmajor(W1hi_f))
    W1lo = _q8(kmajor(W1 * t1 - W1hi_f))
    W2q = _q8(kmajor(W2 * t2))
    W3b = kmajor(W3 * t3).astype(BF)

    def bias_lhsT(b, S):
        bhi = _q8(b * S / 32.0).astype(np.float32)
        blo = _q8((b * S - 32.0 * bhi) / 32.0).astype(np.float32)
        L = np.zeros((128, 2, HID), np.float32)
        L[:32, 0, :] = bhi[None, :]
        L[32:64, 0, :] = blo[None, :]
        return L.astype(E4)

    shared = {
        "yT": (y * s0).T.copy(),
        "Wx": Wx,
        "Wy": Wy,
        "b0s": ((b0 * s0).reshape(KB, 128).T).copy(),
        "W1hi": W1hi,
        "W1lo": W1lo,
        "W2q": W2q,
        "W3b": W3b,
        "bias1L": bias_lhsT(b1, s0 * t1),
        "bias2L": bias_lhsT(b2, s1 * t2),
        "ones8": np.ones((128, 2, PAIR), np.float32).astype(E4),
    }
    in_maps = []
    for c in range(N_CORES):
        im = dict(shared)
        im["xT"] = ((x[c * ROWS : (c + 1) * ROWS] * s0).T).copy()
        in_maps.append(im)
    scales = (
        float(s1 / (s0 * t1)),
        float(s2 / (s1 * t2)),
        float(1.0 / (s2 * t3)),
    )
    return scales, in_maps


def run(trace=False, **inputs):
    scales, in_maps = _prep(inputs)
    if _CACHE.get("scales") != scales:
        _CACHE["nc"] = _build_nc(*scales)
        _CACHE["scales"] = scales
    nc = _CACHE["nc"]
    res = run_bass_kernel_spmd(nc, in_maps, core_ids=list(range(N_CORES)), trace=trace)
    b3 = np.float32(np.asarray(inputs["b3"]).reshape(-1)[0])
    blocks = [r["out"].reshape(ROWS, B) + b3 for r in res.results]
    return np.concatenate(blocks, axis=0).astype(np.float32), res


def _get_nc():
    return _CACHE["nc"]


def kernel(**inputs):
    out, _ = run(trace=False, **inputs)
    return out


# revision 5
# speedup vs baseline: 1.4104x; 1.1636x over previous
"""ConcatCritic pair-grid MLP on 8 TRN2 NeuronCores — fp8 DoubleRow version.

Computes out[i, j] = f(x[i], y[j]) where f is a 3-hidden-layer MLP over the
concatenated pair, decomposed so the first layer is two small projections
summed by broadcast.

Sharding: the B^2 pair grid is split row-wise (x batch) across 8 cores;
y and all MLP parameters are replicated. Each core produces a [B/8, B]
score tile; the host concatenates them. b3 (a scalar) is added on the host.

Numerics: layers 1-2 run on the PE array in fp8e4m3 with DoubleRow perf
mode (two 128-row k-groups per instruction at 0.5 cycles/row). Activations
are stored as single fp8 with power-of-2 scales; W1 is split hi+lo into two
fp8 products to cut its quantization error; W2 is a single fp8 product;
layer 3 runs in bf16. Each layer's bias is folded into one extra DoubleRow
matmul against a constant ones tile (the bias is hi/lo-encoded across 64
lhsT slots), which makes every PSUM drain a pure scale+relu op that DVE and
ACT can both execute, including paired two-bank drains.
"""

import numpy as np
import ml_dtypes

import concourse.bass as bass
import concourse.mybir as mybir
from concourse import bacc
from concourse.bass_utils import run_bass_kernel_spmd
from concourse.tile import TileContext

B = 256
A_DIM = 128
HID = 512
N_CORES = 8
ROWS = B // N_CORES  # 32 x-rows per core
KB = HID // 128  # 4 k-blocks of 128
PAIR = 512  # pairs per tile = 2 x-rows x 256 y-rows
ROWS_PER_TILE = PAIR // B  # 2
N_TILES = ROWS // ROWS_PER_TILE  # 16

F32 = mybir.dt.float32
F32R = mybir.dt.float32r
F8 = mybir.dt.float8e4
BF16 = mybir.dt.bfloat16
E4 = ml_dtypes.float8_e4m3
BF = ml_dtypes.bfloat16
DR = mybir.MatmulPerfMode.DoubleRow

_CACHE = {}


def _build_nc(sc1, sc2, sc3):
    nc = bacc.Bacc()

    xT = nc.declare_dram_parameter("xT", [A_DIM, ROWS], F32R, isOutput=False)
    yT = nc.declare_dram_parameter("yT", [A_DIM, B], F32R, isOutput=False)
    Wx = nc.declare_dram_parameter("Wx", [A_DIM, HID], F32R, isOutput=False)
    Wy = nc.declare_dram_parameter("Wy", [A_DIM, HID], F32R, isOutput=False)
    b0s = nc.declare_dram_parameter("b0s", [128, KB], F32, isOutput=False)
    W1hi = nc.declare_dram_parameter("W1hi", [128, KB, HID], F8, isOutput=False)
    W1lo = nc.declare_dram_parameter("W1lo", [128, KB, HID], F8, isOutput=False)
    W2q = nc.declare_dram_parameter("W2q", [128, KB, HID], F8, isOutput=False)
    W3b = nc.declare_dram_parameter("W3b", [128, KB, 1], BF16, isOutput=False)
    bias1L = nc.declare_dram_parameter("bias1L", [128, 2, HID], F8, isOutput=False)
    bias2L = nc.declare_dram_parameter("bias2L", [128, 2, HID], F8, isOutput=False)
    ones8 = nc.declare_dram_parameter("ones8", [128, 2, PAIR], F8, isOutput=False)
    out = nc.declare_dram_parameter("out", [1, ROWS * B], F32, isOutput=True)

    relu = mybir.ActivationFunctionType.Relu
    copyf = mybir.ActivationFunctionType.Copy
    add = mybir.AluOpType.add
    amax = mybir.AluOpType.max
    mult = mybir.AluOpType.mult

    with TileContext(nc) as tc:
        with (
            tc.tile_pool(name="const", bufs=1) as const,
            tc.tile_pool(name="h0p", bufs=2) as h0p,
            tc.tile_pool(name="h1p", bufs=2) as h1p,
            tc.tile_pool(name="h2p", bufs=2) as h2p,
            tc.tile_pool(name="scp", bufs=4) as scp,
            tc.tile_pool(name="psA", bufs=3, space="PSUM") as psA,
            tc.tile_pool(name="psS", bufs=2, space="PSUM") as psS,
        ):
            # ---- replicated constants -----------------------------------
            W1hi_sb = const.tile([128, KB, HID], F8)
            W1lo_sb = const.tile([128, KB, HID], F8)
            W2q_sb = const.tile([128, KB, HID], F8)
            W3b_sb = const.tile([128, KB, 1], BF16)
            bias1_sb = const.tile([128, 2, HID], F8)
            bias2_sb = const.tile([128, 2, HID], F8)
            ones_sb = const.tile([128, 2, PAIR], F8)
            xT_sb = const.tile([A_DIM, ROWS], F32R)
            yT_sb = const.tile([A_DIM, B], F32R)
            Wx_sb = const.tile([A_DIM, HID], F32R)
            Wy_sb = const.tile([A_DIM, HID], F32R)
            b0s_sb = const.tile([128, KB], F32)

            nc.sync.dma_start(xT_sb[:], xT[:, :])
            nc.sync.dma_start(Wx_sb[:], Wx[:, :])
            nc.sync.dma_start(yT_sb[:], yT[:, :])
            nc.sync.dma_start(Wy_sb[:], Wy[:, :])
            nc.sync.dma_start(b0s_sb[:], b0s[:, :])
            nc.sync.dma_start(W1hi_sb[:], W1hi[:, :, :])
            nc.sync.dma_start(ones_sb[:], ones8[:, :, :])
            nc.sync.dma_start(bias1_sb[:], bias1L[:, :, :])
            nc.sync.dma_start(W1lo_sb[:], W1lo[:, :, :])
            nc.sync.dma_start(W2q_sb[:], W2q[:, :, :])
            nc.sync.dma_start(bias2_sb[:], bias2L[:, :, :])
            nc.sync.dma_start(W3b_sb[:], W3b[:, :, :])

            # ---- input projections (f32r, exact) ------------------------
            # bxT[p, m, i] = (x*s0 @ Wx)^T + b0*s0 (f32; scalar operands)
            # hyT[p, m, j] = (y*s0 @ Wy)^T (bf16)
            bxT = const.tile([128, KB, ROWS], F32)
            hyT = const.tile([128, KB, B], BF16)
            for m in range(KB):
                sl = slice(m * 128, (m + 1) * 128)
                ph = psS.tile([128, PAIR], F32, tag="ps_s", name="ph")[:, :ROWS]
                nc.tensor.matmul(ph, Wx_sb[:, sl], xT_sb[:], start=True, stop=True)
                nc.vector.tensor_scalar(
                    bxT[:, m], ph, b0s_sb[:, m : m + 1], None, add
                )
                ph2 = psS.tile([128, PAIR], F32, tag="ps_s", name="ph2")[:, :B]
                nc.tensor.matmul(ph2, Wy_sb[:, sl], yT_sb[:], start=True, stop=True)
                nc.scalar.copy(out=hyT[:, m], in_=ph2)

            # ---- main pair-tile loop (software pipelined) ---------------
            # iter t runs: layer0(t+1) on DVE/ACT/Pool, L2(t-1), L3(t-2),
            # L1(t) on PE. The PE never waits on a drain in steady state.
            def layer0(t):
                i0 = t * ROWS_PER_TILE
                h0q = h0p.tile([128, KB, PAIR], F8, tag="h0")
                for k in range(KB):
                    for a in range(ROWS_PER_TILE):
                        dst = h0q[:, k, a * B : (a + 1) * B]
                        bxc = bxT[:, k, i0 + a : i0 + a + 1]
                        if k == 3 and a == 0:
                            nc.vector.tensor_scalar(
                                dst, hyT[:, k], bxc, 0.0, add, amax
                            )
                        elif k == 3 and a == 1:
                            nc.scalar.activation(
                                dst, hyT[:, k], relu, bias=bxc, scale=1.0
                            )
                        else:
                            nc.gpsimd.tensor_scalar(
                                dst, hyT[:, k], bxc, 0.0, add, amax
                            )
                return h0q

            def layer1(h0q):
                h1q = h1p.tile([128, KB, PAIR], F8, tag="h1")
                for mp in range(2):
                    pt = psA.tile([128, 2, PAIR], F32, tag="psA", name="pt")
                    for h in range(2):
                        msl = slice((2 * mp + h) * 128, (2 * mp + h + 1) * 128)
                        nc.tensor.matmul(
                            pt[:, h], W1hi_sb[:, 0:2, msl], h0q[:, 0:2, :],
                            start=True, stop=False, perf_mode=DR,
                        )
                        nc.tensor.matmul(
                            pt[:, h], W1hi_sb[:, 2:4, msl], h0q[:, 2:4, :],
                            start=False, stop=False, perf_mode=DR,
                        )
                        nc.tensor.matmul(
                            pt[:, h], W1lo_sb[:, 0:2, msl], h0q[:, 0:2, :],
                            start=False, stop=False, perf_mode=DR,
                        )
                        nc.tensor.matmul(
                            pt[:, h], W1lo_sb[:, 2:4, msl], h0q[:, 2:4, :],
                            start=False, stop=False, perf_mode=DR,
                        )
                        nc.tensor.matmul(
                            pt[:, h], bias1_sb[:, :, msl], ones_sb[:],
                            start=False, stop=True, perf_mode=DR,
                        )
                    dst = h1q[:, 2 * mp : 2 * mp + 2, :]
                    if mp == 0:
                        nc.vector.tensor_scalar(dst, pt, sc1, 0.0, mult, amax)
                    else:
                        nc.scalar.activation(dst, pt, relu, bias=0.0, scale=sc1)
                return h1q

            def layer2(h1q):
                h2q = h2p.tile([128, KB, PAIR], BF16, tag="h2")
                for mp in range(2):
                    pt = psA.tile([128, 2, PAIR], F32, tag="psA", name="pt2")
                    for h in range(2):
                        msl = slice((2 * mp + h) * 128, (2 * mp + h + 1) * 128)
                        nc.tensor.matmul(
                            pt[:, h], W2q_sb[:, 0:2, msl], h1q[:, 0:2, :],
                            start=True, stop=False, perf_mode=DR,
                        )
                        nc.tensor.matmul(
                            pt[:, h], W2q_sb[:, 2:4, msl], h1q[:, 2:4, :],
                            start=False, stop=False, perf_mode=DR,
                        )
                        nc.tensor.matmul(
                            pt[:, h], bias2_sb[:, :, msl], ones_sb[:],
                            start=False, stop=True, perf_mode=DR,
                        )
                    dst = h2q[:, 2 * mp : 2 * mp + 2, :]
                    if mp == 0:
                        nc.vector.tensor_scalar(dst, pt, sc2, 0.0, mult, amax)
                    else:
                        nc.scalar.activation(dst, pt, relu, bias=0.0, scale=sc2)
                return h2q

            def layer3(t, h2q):
                ps3 = psS.tile([128, PAIR], F32, tag="ps_s", name="ps3")[:1]
                for k in range(KB):
                    nc.tensor.matmul(
                        ps3, W3b_sb[:, k], h2q[:, k],
                        start=(k == 0), stop=(k == KB - 1),
                    )
                sc_sb = scp.tile([1, PAIR], F32, tag="sc_sb")
                nc.scalar.activation(sc_sb[:], ps3, copyf, bias=0.0, scale=sc3)
                nc.sync.dma_start(out[:, t * PAIR : (t + 1) * PAIR], sc_sb[:])

            h0s = {0: layer0(0)}
            h1s = {}
            h2s = {}
            for t in range(N_TILES):
                if t + 1 < N_TILES:
                    h0s[t + 1] = layer0(t + 1)
                if t >= 1:
                    h2s[t - 1] = layer2(h1s.pop(t - 1))
                if t >= 2:
                    layer3(t - 2, h2s.pop(t - 2))
                h1s[t] = layer1(h0s.pop(t))
            h2s[N_TILES - 1] = layer2(h1s.pop(N_TILES - 1))
            layer3(N_TILES - 2, h2s.pop(N_TILES - 2))
            layer3(N_TILES - 1, h2s.pop(N_TILES - 1))

    nc.compile()
    return nc


def _q8(a):
    return np.asarray(a, np.float32).astype(E4)


def _p2(m, target=112.0):
    return float(2.0 ** np.floor(np.log2(target / m)))


def _prep(inputs):
    """Host-side quantization; returns (scales, shared in_map, per-core xT)."""
    f = lambda a: np.ascontiguousarray(np.asarray(a), dtype=np.float32)
    x, y = f(inputs["x"]), f(inputs["y"])
    Wx, Wy, b0 = f(inputs["Wx"]), f(inputs["Wy"]), f(inputs["b0"])
    W1, b1 = f(inputs["W1"]), f(inputs["b1"])
    W2, b2 = f(inputs["W2"]), f(inputs["b2"])
    W3 = f(inputs["W3"])

    hx = x @ Wx
    hy = y @ Wy
    h0max = float(np.max(np.max(hx + b0, 0) + np.max(hy, 0)))
    s0 = _p2(h0max)
    # subsampled forward for h1/h2 ranges (16 x-rows), 2x margin
    h0s = np.maximum(hx[::16][:, None, :] + hy[None, :, :] + b0, 0)
    h1s = np.maximum(h0s @ W1 + b1, 0)
    h2s = np.maximum(h1s @ W2 + b2, 0)
    s1 = _p2(float(np.max(h1s)) * 2) * 2
    s2 = _p2(float(np.max(h2s)) * 2) * 2
    t1 = _p2(float(np.max(np.abs(W1))))
    t2 = _p2(float(np.max(np.abs(W2))))
    t3 = _p2(float(np.max(np.abs(W3))))

    def kmajor(W):  # [HID, N] -> [128, KB, N]
        return np.ascontiguousarray(W.reshape(KB, 128, -1).transpose(1, 0, 2))

    W1hi_f = _q8(W1 * t1).astype(np.float32)
    W1hi = _q8(kmajor(W1hi_f))
    W1lo = _q8(kmajor(W1 * t1 - W1hi_f))
    W2q = _q8(kmajor(W2 * t2))
    W3b = kmajor(W3 * t3).astype(BF)

    def bias_lhsT(b, S):
        bhi = _q8(b * S / 32.0).astype(np.float32)
        blo = _q8((b * S - 32.0 * bhi) / 32.0).astype(np.float32)
        L = np.zeros((128, 2, HID), np.float32)
        L[:32, 0, :] = bhi[None, :]
        L[32:64, 0, :] = blo[None, :]
        return L.astype(E4)

    shared = {
        "yT": (y * s0).T.copy(),
        "Wx": Wx,
        "Wy": Wy,
        "b0s": ((b0 * s0).reshape(KB, 128).T).copy(),
        "W1hi": W1hi,
        "W1lo": W1lo,
        "W2q": W2q,
        "W3b": W3b,
        "bias1L": bias_lhsT(b1, s0 * t1),
        "bias2L": bias_lhsT(b2, s1 * t2),
        "ones8": np.ones((128, 2, PAIR), np.float32).astype(E4),
    }
    in_maps = []
    for c in range(N_CORES):
        im = dict(shared)
        im["xT"] = ((x[c * ROWS : (c + 1) * ROWS] * s0).T).copy()
        in_maps.append(im)
    scales = (
        float(s1 / (s0 * t1)),
        float(s2 / (s1 * t2)),
        float(1.0 / (s2 * t3)),
    )
    return scales, in_maps


def run(trace=False, **inputs):
    scales, in_maps = _prep(inputs)
    if _CACHE.get("scales") != scales:
        _CACHE["nc"] = _build_nc(*scales)
        _CACHE["scales"] = scales
    nc = _CACHE["nc"]
    res = run_bass_kernel_spmd(nc, in_maps, core_ids=list(range(N_CORES)), trace=trace)
    b3 = np.float32(np.asarray(inputs["b3"]).reshape(-1)[0])
    blocks = [r["out"].reshape(ROWS, B) + b3 for r in res.results]
    return np.concatenate(blocks, axis=0).astype(np.float32), res


def _get_nc():
    return _CACHE["nc"]


def kernel(**inputs):
    out, _ = run(trace=False, **inputs)
    return out


# revision 6
# speedup vs baseline: 1.5575x; 1.1043x over previous
"""ConcatCritic pair-grid MLP on 8 TRN2 NeuronCores — fp8 DoubleRow version.

Computes out[i, j] = f(x[i], y[j]) where f is a 3-hidden-layer MLP over the
concatenated pair, decomposed so the first layer is two small projections
summed by broadcast.

Sharding: the B^2 pair grid is split row-wise (x batch) across 8 cores;
y and all MLP parameters are replicated. Each core produces a [B/8, B]
score tile; the host concatenates them. b3 (a scalar) is added on the host.

Numerics: layers 1-2 run on the PE array in fp8e4m3 with DoubleRow perf
mode (two 128-row k-groups per instruction at 0.5 cycles/row). Activations
are stored as single fp8 with power-of-2 scales; W1 is split hi+lo into two
fp8 products to cut its quantization error; W2 is a single fp8 product;
layer 3 runs in bf16. Each layer's bias is folded into one extra DoubleRow
matmul against a constant ones tile (the bias is hi/lo-encoded across 64
lhsT slots), which makes every PSUM drain a pure scale+relu op that DVE and
ACT can both execute, including paired two-bank drains.
"""

import numpy as np
import ml_dtypes

import concourse.bass as bass
import concourse.mybir as mybir
from concourse import bacc
from concourse.bass_utils import run_bass_kernel_spmd
from concourse.tile import TileContext

B = 256
A_DIM = 128
HID = 512
N_CORES = 8
ROWS = B // N_CORES  # 32 x-rows per core
KB = HID // 128  # 4 k-blocks of 128
PAIR = 512  # pairs per tile = 2 x-rows x 256 y-rows
ROWS_PER_TILE = PAIR // B  # 2
N_TILES = ROWS // ROWS_PER_TILE  # 16

F32 = mybir.dt.float32
F32R = mybir.dt.float32r
F8 = mybir.dt.float8e4
BF16 = mybir.dt.bfloat16
E4 = ml_dtypes.float8_e4m3
BF = ml_dtypes.bfloat16
DR = mybir.MatmulPerfMode.DoubleRow

_CACHE = {}


def _build_nc(sc1, sc2, sc3):
    nc = bacc.Bacc()

    xT = nc.declare_dram_parameter("xT", [A_DIM, ROWS], F32R, isOutput=False)
    yT = nc.declare_dram_parameter("yT", [A_DIM, B], F32R, isOutput=False)
    Wx = nc.declare_dram_parameter("Wx", [A_DIM, HID], F32R, isOutput=False)
    Wy = nc.declare_dram_parameter("Wy", [A_DIM, HID], F32R, isOutput=False)
    b0s = nc.declare_dram_parameter("b0s", [128, KB], F32, isOutput=False)
    W1hi = nc.declare_dram_parameter("W1hi", [128, KB, HID], F8, isOutput=False)
    W1lo = nc.declare_dram_parameter("W1lo", [128, KB, HID], F8, isOutput=False)
    W2q = nc.declare_dram_parameter("W2q", [128, KB, HID], F8, isOutput=False)
    W3b = nc.declare_dram_parameter("W3b", [128, KB, 1], BF16, isOutput=False)
    bias1L = nc.declare_dram_parameter("bias1L", [128, 2, HID], F8, isOutput=False)
    bias2L = nc.declare_dram_parameter("bias2L", [128, 2, HID], F8, isOutput=False)
    ones8 = nc.declare_dram_parameter("ones8", [128, 2, PAIR], F8, isOutput=False)
    out = nc.declare_dram_parameter("out", [1, ROWS * B], F32, isOutput=True)

    relu = mybir.ActivationFunctionType.Relu
    copyf = mybir.ActivationFunctionType.Copy
    add = mybir.AluOpType.add
    amax = mybir.AluOpType.max
    mult = mybir.AluOpType.mult

    with TileContext(nc) as tc:
        with (
            tc.tile_pool(name="const", bufs=1) as const,
            tc.tile_pool(name="h0p", bufs=2) as h0p,
            tc.tile_pool(name="h1p", bufs=2) as h1p,
            tc.tile_pool(name="h2p", bufs=2) as h2p,
            tc.tile_pool(name="scp", bufs=4) as scp,
            tc.tile_pool(name="psA", bufs=2, space="PSUM") as psA,
            tc.tile_pool(name="psB", bufs=2, space="PSUM") as psB,
        ):
            # ---- replicated constants -----------------------------------
            W1hi_sb = const.tile([128, KB, HID], F8)
            W1lo_sb = const.tile([128, KB, HID], F8)
            W2q_sb = const.tile([128, KB, HID], F8)
            W3b_sb = const.tile([128, KB, 1], BF16)
            bias1_sb = const.tile([128, 2, HID], F8)
            bias2_sb = const.tile([128, 2, HID], F8)
            ones_sb = const.tile([128, 2, PAIR], F8)
            xT_sb = const.tile([A_DIM, ROWS], F32R)
            yT_sb = const.tile([A_DIM, B], F32R)
            Wx_sb = const.tile([A_DIM, HID], F32R)
            Wy_sb = const.tile([A_DIM, HID], F32R)
            b0s_sb = const.tile([128, KB], F32)

            nc.sync.dma_start(xT_sb[:], xT[:, :])
            nc.sync.dma_start(Wx_sb[:], Wx[:, :])
            nc.sync.dma_start(yT_sb[:], yT[:, :])
            nc.sync.dma_start(Wy_sb[:], Wy[:, :])
            nc.sync.dma_start(b0s_sb[:], b0s[:, :])
            nc.sync.dma_start(W1hi_sb[:], W1hi[:, :, :])
            nc.sync.dma_start(ones_sb[:], ones8[:, :, :])
            nc.sync.dma_start(bias1_sb[:], bias1L[:, :, :])
            nc.sync.dma_start(W1lo_sb[:], W1lo[:, :, :])
            nc.sync.dma_start(W2q_sb[:], W2q[:, :, :])
            nc.sync.dma_start(bias2_sb[:], bias2L[:, :, :])
            nc.sync.dma_start(W3b_sb[:], W3b[:, :, :])

            # ---- input projections (f32r, exact) ------------------------
            # bxT[p, m, i] = (x*s0 @ Wx)^T + b0*s0 (f32; scalar operands)
            # hyT[p, m, j] = (y*s0 @ Wy)^T (bf16)
            bxT = const.tile([128, KB, ROWS], F32)
            hyT = const.tile([128, KB, B], BF16)
            for m in range(KB):
                sl = slice(m * 128, (m + 1) * 128)
                ph = psB.tile([128, 2, PAIR], F32, tag="psB", name="ph")[:, 0, :ROWS]
                nc.tensor.matmul(ph, Wx_sb[:, sl], xT_sb[:], start=True, stop=True)
                nc.vector.tensor_scalar(
                    bxT[:, m], ph, b0s_sb[:, m : m + 1], None, add
                )
                ph2 = psB.tile([128, 2, PAIR], F32, tag="psB", name="ph2")[:, 0, :B]
                nc.tensor.matmul(ph2, Wy_sb[:, sl], yT_sb[:], start=True, stop=True)
                nc.scalar.copy(out=hyT[:, m], in_=ph2)

            # ---- main pair-tile loop (software pipelined) ---------------
            # iter t runs: layer0(t+1) on DVE/ACT/Pool, L2(t-1), L3(t-2),
            # L1(t) on PE. The PE never waits on a drain in steady state.
            def layer0(t):
                i0 = t * ROWS_PER_TILE
                h0q = h0p.tile([128, KB, PAIR], F8, tag="h0")
                for k in range(KB):
                    for a in range(ROWS_PER_TILE):
                        dst = h0q[:, k, a * B : (a + 1) * B]
                        bxc = bxT[:, k, i0 + a : i0 + a + 1]
                        if k == 3 and a == 0:
                            nc.vector.tensor_scalar(
                                dst, hyT[:, k], bxc, 0.0, add, amax
                            )
                        elif k == 3 and a == 1:
                            nc.scalar.activation(
                                dst, hyT[:, k], relu, bias=bxc, scale=1.0
                            )
                        else:
                            nc.gpsimd.tensor_scalar(
                                dst, hyT[:, k], bxc, 0.0, add, amax
                            )
                return h0q

            def layer1(h0q):
                h1q = h1p.tile([128, KB, PAIR], F8, tag="h1")
                for mp in range(2):
                    pt = psA.tile([128, 2, PAIR], F32, tag="psA", name="pt")
                    for h in range(2):
                        msl = slice((2 * mp + h) * 128, (2 * mp + h + 1) * 128)
                        nc.tensor.matmul(
                            pt[:, h], W1hi_sb[:, 0:2, msl], h0q[:, 0:2, :],
                            start=True, stop=False, perf_mode=DR,
                        )
                        nc.tensor.matmul(
                            pt[:, h], W1hi_sb[:, 2:4, msl], h0q[:, 2:4, :],
                            start=False, stop=False, perf_mode=DR,
                        )
                        nc.tensor.matmul(
                            pt[:, h], W1lo_sb[:, 0:2, msl], h0q[:, 0:2, :],
                            start=False, stop=False, perf_mode=DR,
                        )
                        nc.tensor.matmul(
                            pt[:, h], W1lo_sb[:, 2:4, msl], h0q[:, 2:4, :],
                            start=False, stop=False, perf_mode=DR,
                        )
                        nc.tensor.matmul(
                            pt[:, h], bias1_sb[:, :, msl], ones_sb[:],
                            start=False, stop=True, perf_mode=DR,
                        )
                    dst = h1q[:, 2 * mp : 2 * mp + 2, :]
                    if mp == 0:
                        nc.vector.tensor_scalar(dst, pt, sc1, 0.0, mult, amax)
                    else:
                        nc.scalar.activation(dst, pt, relu, bias=0.0, scale=sc1)
                return h1q

            def layer2(h1q):
                h2q = h2p.tile([128, KB, PAIR], BF16, tag="h2")
                for mp in range(2):
                    pt = psB.tile([128, 2, PAIR], F32, tag="psB", name="pt2")
                    for h in range(2):
                        msl = slice((2 * mp + h) * 128, (2 * mp + h + 1) * 128)
                        nc.tensor.matmul(
                            pt[:, h], W2q_sb[:, 0:2, msl], h1q[:, 0:2, :],
                            start=True, stop=False, perf_mode=DR,
                        )
                        nc.tensor.matmul(
                            pt[:, h], W2q_sb[:, 2:4, msl], h1q[:, 2:4, :],
                            start=False, stop=False, perf_mode=DR,
                        )
                        nc.tensor.matmul(
                            pt[:, h], bias2_sb[:, :, msl], ones_sb[:],
                            start=False, stop=True, perf_mode=DR,
                        )
                    dst = h2q[:, 2 * mp : 2 * mp + 2, :]
                    if mp == 0:
                        nc.vector.tensor_scalar(dst, pt, sc2, 0.0, mult, amax)
                    else:
                        nc.scalar.activation(dst, pt, relu, bias=0.0, scale=sc2)
                return h2q

            def layer3(t, h2q):
                ps3 = psA.tile([128, 2, PAIR], F32, tag="psA", name="ps3")[:1, 0]
                for k in range(KB):
                    nc.tensor.matmul(
                        ps3, W3b_sb[:, k], h2q[:, k],
                        start=(k == 0), stop=(k == KB - 1),
                    )
                sc_sb = scp.tile([1, PAIR], F32, tag="sc_sb")
                nc.scalar.activation(sc_sb[:], ps3, copyf, bias=0.0, scale=sc3)
                nc.sync.dma_start(out[:, t * PAIR : (t + 1) * PAIR], sc_sb[:])

            h0s = {0: layer0(0)}
            h1s = {}
            h2s = {}
            for t in range(N_TILES):
                if t + 1 < N_TILES:
                    h0s[t + 1] = layer0(t + 1)
                h1s[t] = layer1(h0s.pop(t))
                if t >= 1:
                    h2s[t - 1] = layer2(h1s.pop(t - 1))
                if t >= 2:
                    layer3(t - 2, h2s.pop(t - 2))
            h2s[N_TILES - 1] = layer2(h1s.pop(N_TILES - 1))
            layer3(N_TILES - 2, h2s.pop(N_TILES - 2))
            layer3(N_TILES - 1, h2s.pop(N_TILES - 1))

    nc.compile()
    return nc


def _q8(a):
    return np.asarray(a, np.float32).astype(E4)


def _p2(m, target=112.0):
    return float(2.0 ** np.floor(np.log2(target / m)))


def _prep(inputs):
    """Host-side quantization; returns (scales, shared in_map, per-core xT)."""
    f = lambda a: np.ascontiguousarray(np.asarray(a), dtype=np.float32)
    x, y = f(inputs["x"]), f(inputs["y"])
    Wx, Wy, b0 = f(inputs["Wx"]), f(inputs["Wy"]), f(inputs["b0"])
    W1, b1 = f(inputs["W1"]), f(inputs["b1"])
    W2, b2 = f(inputs["W2"]), f(inputs["b2"])
    W3 = f(inputs["W3"])

    hx = x @ Wx
    hy = y @ Wy
    h0max = float(np.max(np.max(hx + b0, 0) + np.max(hy, 0)))
    s0 = _p2(h0max)
    # subsampled forward for h1/h2 ranges (16 x-rows), 2x margin
    h0s = np.maximum(hx[::16][:, None, :] + hy[None, :, :] + b0, 0)
    h1s = np.maximum(h0s @ W1 + b1, 0)
    h2s = np.maximum(h1s @ W2 + b2, 0)
    s1 = _p2(float(np.max(h1s)) * 2) * 2
    s2 = _p2(float(np.max(h2s)) * 2) * 2
    t1 = _p2(float(np.max(np.abs(W1))))
    t2 = _p2(float(np.max(np.abs(W2))))
    t3 = _p2(float(np.max(np.abs(W3))))

    def kmajor(W):  # [HID, N] -> [128, KB, N]
        return np.ascontiguousarray(W.reshape(KB, 128, -1).transpose(1, 0, 2))

    W1hi_f = _q8(W1 * t1).astype(np.float32)
    W1hi = _q8(kmajor(W1hi_f))
    W1lo = _q8(kmajor(W1 * t1 - W1hi_f))
    W2q = _q8(kmajor(W2 * t2))
    W3b = kmajor(W3 * t3).astype(BF)

    def bias_lhsT(b, S):
        bhi = _q8(b * S / 32.0).astype(np.float32)
        blo = _q8((b * S - 32.0 * bhi) / 32.0).astype(np.float32)
        L = np.zeros((128, 2, HID), np.float32)
        L[:32, 0, :] = bhi[None, :]
        L[32:64, 0, :] = blo[None, :]
        return L.astype(E4)

    shared = {
        "yT": (y * s0).T.copy(),
        "Wx": Wx,
        "Wy": Wy,
        "b0s": ((b0 * s0).reshape(KB, 128).T).copy(),
        "W1hi": W1hi,
        "W1lo": W1lo,
        "W2q": W2q,
        "W3b": W3b,
        "bias1L": bias_lhsT(b1, s0 * t1),
        "bias2L": bias_lhsT(b2, s1 * t2),
        "ones8": np.ones((128, 2, PAIR), np.float32).astype(E4),
    }
    in_maps = []
    for c in range(N_CORES):
        im = dict(shared)
        im["xT"] = ((x[c * ROWS : (c + 1) * ROWS] * s0).T).copy()
        in_maps.append(im)
    scales = (
        float(s1 / (s0 * t1)),
        float(s2 / (s1 * t2)),
        float(1.0 / (s2 * t3)),
    )
    return scales, in_maps


def run(trace=False, **inputs):
    scales, in_maps = _prep(inputs)
    if _CACHE.get("scales") != scales:
        _CACHE["nc"] = _build_nc(*scales)
        _CACHE["scales"] = scales
    nc = _CACHE["nc"]
    res = run_bass_kernel_spmd(nc, in_maps, core_ids=list(range(N_CORES)), trace=trace)
    b3 = np.float32(np.asarray(inputs["b3"]).reshape(-1)[0])
    blocks = [r["out"].reshape(ROWS, B) + b3 for r in res.results]
    return np.concatenate(blocks, axis=0).astype(np.float32), res


def _get_nc():
    return _CACHE["nc"]


def kernel(**inputs):
    out, _ = run(trace=False, **inputs)
    return out
